# revision 44
# baseline (speedup 1.0000x reference)
"""Trainium2 Bass kernel for MultiHeadLatentAttention (B=4, S=8192, E=2048,
H=16, latent=head_dim=128), SPMD over 8 NeuronCores.

Math (reference):
    q = rope(X_q @ Wq + bq); k = rope(X_k @ Wk + bk); v = X_v @ Wv + bv
    reshape folds seq into heads: q[b,h,s',d] = q_lat[b, 16*s'+h, d], S'=512
    attn per (b,h): softmax(q k^T / sqrt(128)) @ v   -> out @ Wo + bo

Host-side transforms (exact, no approximation):
  * rope here is position-independent (freqs have a singleton seq axis), so
    rope(x) == x @ R for a fixed 128x128 2-diagonal matrix R. We fold R (and
    the 1/sqrt(128) score scale) into Wq / Wk.
  * mask is all ones -> no-op.
  * Sharding: 2 heads per core x all 4 batches: each core projects exactly the
    input rows its heads need (zero redundant FLOPs), runs 8 [512x512]
    attentions, and computes a partial out-projection over its 256 latent
    channels.  Host sums the 8 partials (+ bo).
  * x shipped partition-major pre-transposed so every DMA has 16KB contiguous
    partition lines (large packets ~ peak DMA engine rate).

Device structure per core: a 4-stage pipeline over batches.  For each batch
quarter: project q/k/v (PE, accumulating over 16 E-chunks), PE-transpose v,
2 attention groups (scores^T -> exp on ACT -> ones-matmul denominator ->
PV, normalization fused into the PSUM drain), partial out-projection.
Per-batch tiles let the Tile scheduler overlap batch b's attention with
batch b+1's input DMA.
"""

import os

import numpy as np

import concourse.bass as bass
import concourse.mybir as mybir
import concourse.tile as tile
from concourse import bacc
from concourse.bass_utils import run_bass_kernel_spmd
from concourse.masks import make_identity

B, S, E, H, HD = 4, 8192, 2048, 16, 128
SP = S // H            # 512 folded sequence length
NCORES = 8
HPC = H // NCORES      # heads per core = 2
NG = B * HPC           # attention groups per core = 8
ROWS = NG * SP         # projection rows per core = 4096
BROWS = ROWS // B      # rows per batch quarter = 1024
OROWS = B * SP         # output rows = 2048
KC = E // 128          # contraction chunks = 16
JT = SP // 128         # 128-blocks per group = 4
F32 = mybir.dt.float32

# matmul mode: 'f32' (full precision), 'f32r' (1 cyc/row, tf32-like),
# 'bf16', 'f16' (fp16 inputs/projections + f32r attention — halves the DMA
# volume, which is the roofline, at ~6e-4 relative error).
MM_MODE = os.environ.get("MLA_MM_MODE", "f16")

_CACHE = {}
LAST_RESULTS = None  # BassKernelResults of the most recent run (for profiling)


def _build(mm_mode, with_bias):
    # x_dt: dtype of the streamed inputs + projection weights (sets the DMA
    # byte volume).  att_dt: dtype of on-chip latents / attention / out-proj
    # operands.  float32r tiles must be written as float32r end-to-end (BIR
    # rounding rule), hence dtype-typed tiles rather than bitcasts.
    x_dt, att_dt = {
        "f32": (F32, F32),
        "f32r": (mybir.dt.float32r, mybir.dt.float32r),
        "bf16": (mybir.dt.bfloat16, mybir.dt.bfloat16),
        "f16": (mybir.dt.float16, mybir.dt.float32r),
    }[mm_mode]
    # out-projection side (AT, Wo, out partials): fp16 in f16 mode — the
    # partials are summed across cores on the host, so fp16 output
    # quantization stays ~5e-4 while saving ~9MB of DMA per core.
    o_dt = mybir.dt.float16 if mm_mode == "f16" else att_dt
    oo_dt = mybir.dt.float16 if mm_mode == "f16" else F32
    # exp/PV operand side: fp16 halves the E-tile SBUF footprint (deeper
    # input prefetch) at ~1e-4 extra error on the softmax weights.
    e_dt = mybir.dt.float16 if mm_mode == "f16" else att_dt

    nc = bacc.Bacc("TRN2", target_bir_lowering=False, debug=False,
                   num_devices=NCORES)
    # x layout: [128, B, KC, BROWS] flattened — a per-batch 8-chunk strip is
    # one DMA with 16KB+ contiguous partition lines.
    xq = nc.dram_tensor("xq", [128, KC * ROWS], x_dt, kind="ExternalInput")
    xk = nc.dram_tensor("xk", [128, KC * ROWS], x_dt, kind="ExternalInput")
    xv = nc.dram_tensor("xv", [128, KC * ROWS], x_dt, kind="ExternalInput")
    wq = nc.dram_tensor("wq", [128, KC * HD], x_dt, kind="ExternalInput")
    wk = nc.dram_tensor("wk", [128, KC * HD], x_dt, kind="ExternalInput")
    wv = nc.dram_tensor("wv", [128, KC * HD], x_dt, kind="ExternalInput")
    wo = nc.dram_tensor("wo", [128, HPC * E], o_dt, kind="ExternalInput")
    if with_bias:
        bqkv = nc.dram_tensor("bqkv", [3, HD], F32, kind="ExternalInput")
    out = nc.dram_tensor("out", [OROWS, E], oo_dt, kind="ExternalOutput")

    with tile.TileContext(nc) as tc:
        with tc.tile_pool(name="persist", bufs=1) as persist, \
             tc.tile_pool(name="lat", bufs=3) as lat, \
             tc.tile_pool(name="work", bufs=2) as work, \
             tc.tile_pool(name="xin", bufs=7) as xin, \
             tc.tile_pool(name="psproj", bufs=2, space="PSUM") as psproj, \
             tc.tile_pool(name="pss", bufs=2, space="PSUM") as pss, \
             tc.tile_pool(name="pssum", bufs=1, space="PSUM") as pssum, \
             tc.tile_pool(name="pso", bufs=1, space="PSUM") as pso, \
             tc.tile_pool(name="psod", bufs=2, space="PSUM") as psod:
            # all-ones stationary: ones^T @ E gives the softmax denominator
            # replicated across all 128 output partitions (no cross-partition
            # broadcast needed).  memset/affine_select only handle plain
            # dtypes -> build in f32 and convert.
            ones_t = persist.tile([128, 128], e_dt)
            ident = persist.tile([128, 128], att_dt)
            if e_dt == F32:
                nc.gpsimd.memset(ones_t[:], 1.0)
                make_identity(nc, ident[:])
            else:
                scratch = persist.tile([128, 128], F32)
                nc.gpsimd.memset(scratch[:], 1.0)
                nc.vector.tensor_copy(ones_t[:], scratch[:])
                make_identity(nc, scratch[:])
                nc.vector.tensor_copy(ident[:], scratch[:])
            if with_bias:
                bias_sb = persist.tile([128, 3], F32)
                nc.sync.dma_start(bias_sb[:], bqkv[:].rearrange("t l -> l t"))

            w_sb = {}
            for name, w in (("q", wq), ("k", wk), ("v", wv)):
                t = persist.tile([128, KC, HD], x_dt, tag=f"w_{name}",
                                 name=f"w_{name}")
                nc.sync.dma_start(t[:], w[:].rearrange("p (c l) -> p c l",
                                                       c=KC))
                w_sb[name] = t
            wo_sb = persist.tile([128, HPC, E], o_dt)

            # x layout: [128, NG, KC, SP] flattened — one 2MB strip per
            # (tensor, group) with 16KB contiguous partition lines.
            xr = {name: src[:].rearrange("p (g c r) -> p g c r", g=NG, c=KC)
                  for name, src in (("q", xq), ("k", xk), ("v", xv))}

            qTs, kTs, vNs, ATs = {}, {}, {}, {}

            def proj_stage(g):
                """Project q/k/v for group g (rows g*512..), transpose v."""
                lats = {}
                for ti, name in enumerate(("q", "k", "v")):
                    dst = lat.tile([128, SP], att_dt, tag=f"{name}T",
                                   name=f"{name}T_{g}")
                    lats[name] = dst
                    ps = psproj.tile([128, SP], F32, tag="proj",
                                     name=f"ps_{name}_{g}")
                    xs = xin.tile([128, KC, SP], x_dt, tag="xstrip",
                                  name=f"xs_{name}_{g}")
                    # group 0: split the strip load so the first matmuls start
                    # after 1/4 of the data instead of the full 2MB.
                    npieces = 4 if g == 0 else 1
                    cper = KC // npieces
                    for p0 in range(0, KC, cper):
                        nc.sync.dma_start(xs[:, p0:p0 + cper],
                                          xr[name][:, g, p0:p0 + cper])
                    for c in range(KC):
                        nc.tensor.matmul(ps[:], w_sb[name][:, c], xs[:, c],
                                         start=(c == 0), stop=(c == KC - 1))
                    if with_bias:
                        nc.vector.tensor_scalar_add(dst[:], ps[:],
                                                    bias_sb[:, ti:ti + 1])
                    else:
                        nc.vector.tensor_copy(dst[:], ps[:])
                qTs[g], kTs[g] = lats["q"], lats["k"]
                vN = lat.tile([128, JT, HD], e_dt, tag="vN", name=f"vN_{g}")
                vNs[g] = vN
                for j in range(JT):
                    pt = psod.tile([128, 128], att_dt, tag="od",
                                   name=f"tr_{g}_{j}")
                    nc.tensor.transpose(pt[:], lats["v"][:, j * 128:(j + 1) * 128],
                                        ident[:])
                    nc.vector.tensor_copy(vN[:, j], pt[:])

            def attn_stage(g):
                b, hl = divmod(g, HPC)
                if hl == 0:
                    ATs[b] = lat.tile([128, HPC, SP], o_dt, tag="AT",
                                      name=f"AT_{b}")
                Esb = work.tile([128, JT, SP], e_dt, tag="E", name=f"E_{g}")
                for j in range(JT):
                    sp = pss.tile([128, SP], F32, tag="S", name=f"S_{g}_{j}")
                    nc.tensor.matmul(sp[:], kTs[g][:, j * 128:(j + 1) * 128],
                                     qTs[g][:], start=True, stop=True)
                    nc.scalar.activation(Esb[:, j], sp[:],
                                         mybir.ActivationFunctionType.Exp)
                sum_ps = pssum.tile([128, SP], F32, tag="sum", name=f"sum_{g}")
                for j in range(JT):
                    nc.tensor.matmul(sum_ps[:], ones_t[:], Esb[:, j],
                                     start=(j == 0), stop=(j == JT - 1))
                o_ps = pso.tile([128, SP], F32, tag="O", name=f"O_{g}")
                for j in range(JT):
                    nc.tensor.matmul(o_ps[:], vNs[g][:, j], Esb[:, j],
                                     start=(j == 0), stop=(j == JT - 1))
                rec_b = work.tile([128, SP], F32, tag="recb", name=f"rec_{g}")
                # ~51 ULP, ~5x faster than reciprocal() — this op sits on the
                # critical path between the denominator matmul and PV drain.
                # Inputs are softmax sums (>= 1), so no edge cases.
                nc.vector.reciprocal_approx_fast(rec_b[:], sum_ps[:])
                nc.vector.tensor_tensor(ATs[b][:, hl], o_ps[:], rec_b[:],
                                        op=mybir.AluOpType.mult)
                del qTs[g], kTs[g], vNs[g]

            def out_stage(b):
                AT = ATs[b]
                for rto in range(SP // 128):
                    ot = work.tile([128, E], oo_dt, tag="ot",
                                   name=f"ot_{b}_{rto}")
                    for n in range(E // 512):
                        ps = psod.tile([128, 512], F32, tag="od",
                                       name=f"od_{b}_{rto}_{n}")
                        for hl in range(HPC):
                            nc.tensor.matmul(
                                ps[:], AT[:, hl, rto * 128:(rto + 1) * 128],
                                wo_sb[:, hl, n * 512:(n + 1) * 512],
                                start=(hl == 0), stop=(hl == HPC - 1))
                        # drain on ACT: DVE is the busier epilogue engine
                        nc.scalar.copy(ot[:, n * 512:(n + 1) * 512], ps[:])
                    r0 = b * SP + rto * 128
                    nc.sync.dma_start(out[r0:r0 + 128, :], ot[:])
                del ATs[b]

            # software-pipelined emission: projections run two groups ahead
            # of attention so the input-strip DMA never starves; the serial
            # tail is one group's attention + one out-projection.
            proj_stage(0)
            proj_stage(1)
            nc.sync.dma_start(wo_sb[:],
                              wo[:].rearrange("p (h e) -> p h e", h=HPC))
            for g in range(NG):
                attn_stage(g)
                if g + 2 < NG:
                    proj_stage(g + 2)
                if g % 2 == 1:
                    out_stage(g // 2)

    nc.compile()
    return nc


def _rope_matrix():
    h2 = HD // 2
    freqs = 1.0 / (10000.0 ** (np.arange(0, HD, 2, dtype=np.float64) / HD))
    sin, cos = np.sin(freqs), np.cos(freqs)
    R = np.zeros((HD, HD), np.float64)
    i = np.arange(h2)
    R[i, i] = cos
    R[i + h2, i] = -sin
    R[i + h2, i + h2] = cos
    R[i, i + h2] = sin
    return R


def kernel(query, key, value, attn_mask, Wq, bq, Wk, bk, Wv, bv, Wo, bo,
           _trace=False):
    global LAST_RESULTS
    # inputs may arrive as jax arrays — coerce to host numpy first
    query, key, value = np.asarray(query), np.asarray(key), np.asarray(value)
    Wq, bq = np.asarray(Wq), np.asarray(bq)
    Wk, bk = np.asarray(Wk), np.asarray(bk)
    Wv, bv = np.asarray(Wv), np.asarray(bv)
    Wo, bo = np.asarray(Wo), np.asarray(bo)
    mm_mode = MM_MODE
    io_np = np.dtype("float32")
    wo_np = np.dtype("float32")
    if mm_mode == "bf16":
        import ml_dtypes
        io_np = np.dtype(ml_dtypes.bfloat16)
        wo_np = io_np
    elif mm_mode == "f16":
        io_np = np.dtype("float16")
        wo_np = io_np

    R = _rope_matrix()
    scale = 1.0 / np.sqrt(np.float64(HD))
    wq_eff = (Wq.astype(np.float64) @ R * scale).astype(io_np)
    wk_eff = (Wk.astype(np.float64) @ R).astype(io_np)
    wv_eff = Wv.astype(io_np)
    bq_eff = (bq.astype(np.float64) @ R * scale).astype(np.float32)
    bk_eff = (bk.astype(np.float64) @ R).astype(np.float32)
    bv_eff = bv.astype(np.float32)
    with_bias = bool(np.any(bq_eff) or np.any(bk_eff) or np.any(bv_eff))

    key_ = (mm_mode, with_bias)
    if key_ not in _CACHE:
        _CACHE[key_] = _build(mm_mode, with_bias)
    nc = _CACHE[key_]

    # [B,S,E] -> [E, B, H, SP]; s = s'*H + h so reshape(B, SP, H, E) puts the
    # folded position s' on axis 1 and the head on axis 2.
    def fold(x):
        return np.ascontiguousarray(
            x.reshape(B, SP, H, E).transpose(3, 0, 2, 1).astype(io_np))

    fq, fk, fv = fold(query), fold(key), fold(value)
    wo_r = Wo.reshape(H, HD, E)

    def pmajor(xc):
        # [E, ROWS(b,hl,s')] -> [128, NG, KC, SP] flattened: partition is the
        # inner 128 of the E-chunk; per-group strips contiguous (16KB lines).
        return np.ascontiguousarray(
            xc.reshape(KC, 128, NG, SP).transpose(1, 2, 0, 3)
        ).reshape(128, KC * ROWS)

    # pre-lay weights in SBUF order ([128 partitions, ...]).
    def sb_layout_w(w_eff):  # [E, HD] -> [128, KC*HD]
        return np.ascontiguousarray(
            w_eff.reshape(KC, 128, HD).transpose(1, 0, 2).reshape(128, KC * HD))

    wq_sb, wk_sb, wv_sb = map(sb_layout_w, (wq_eff, wk_eff, wv_eff))

    in_maps = []
    for c in range(NCORES):
        h0 = HPC * c
        wo_c = wo_r[h0:h0 + HPC].astype(wo_np)  # [HPC, HD, E]
        m = {
            "xq": pmajor(fq[:, :, h0:h0 + HPC, :].reshape(E, ROWS)),
            "xk": pmajor(fk[:, :, h0:h0 + HPC, :].reshape(E, ROWS)),
            "xv": pmajor(fv[:, :, h0:h0 + HPC, :].reshape(E, ROWS)),
            "wq": wq_sb, "wk": wk_sb, "wv": wv_sb,
            "wo": np.ascontiguousarray(
                wo_c.transpose(1, 0, 2).reshape(128, HPC * E)),
        }
        if with_bias:
            m["bqkv"] = np.stack([bq_eff, bk_eff, bv_eff])
        in_maps.append(m)

    kwargs = {}
    if _trace:
        kwargs = dict(trace=True, trace_cores=list(range(NCORES)))
    res = run_bass_kernel_spmd(nc, in_maps, core_ids=list(range(NCORES)),
                               **kwargs)
    LAST_RESULTS = res

    total = res.results[0]["out"].astype(np.float64)
    for c in range(1, NCORES):
        total += res.results[c]["out"]
    total += bo.astype(np.float64)
    return total.reshape(B, SP, E).astype(np.float32)


# revision 45
# speedup vs baseline: 1.0814x; 1.0814x over previous
"""Trainium2 Bass kernel for MultiHeadLatentAttention (B=4, S=8192, E=2048,
H=16, latent=head_dim=128), SPMD over 8 NeuronCores.

Math (reference):
    q = rope(X_q @ Wq + bq); k = rope(X_k @ Wk + bk); v = X_v @ Wv + bv
    reshape folds seq into heads: q[b,h,s',d] = q_lat[b, 16*s'+h, d], S'=512
    attn per (b,h): softmax(q k^T / sqrt(128)) @ v   -> out @ Wo + bo

Host-side transforms (exact, no approximation):
  * rope here is position-independent (freqs have a singleton seq axis), so
    rope(x) == x @ R for a fixed 128x128 2-diagonal matrix R. We fold R (and
    the 1/sqrt(128) score scale) into Wq / Wk.
  * mask is all ones -> no-op.
  * Sharding: 2 heads per core x all 4 batches: each core projects exactly the
    input rows its heads need (zero redundant FLOPs), runs 8 [512x512]
    attentions, and computes a partial out-projection over its 256 latent
    channels.  Host sums the 8 partials (+ bo).
  * x shipped partition-major pre-transposed so every DMA has 16KB contiguous
    partition lines (large packets ~ peak DMA engine rate).

Device structure per core: a 4-stage pipeline over batches.  For each batch
quarter: project q/k/v (PE, accumulating over 16 E-chunks), PE-transpose v,
2 attention groups (scores^T -> exp on ACT -> ones-matmul denominator ->
PV, normalization fused into the PSUM drain), partial out-projection.
Per-batch tiles let the Tile scheduler overlap batch b's attention with
batch b+1's input DMA.
"""

import os

import numpy as np

import concourse.bass as bass
import concourse.mybir as mybir
import concourse.tile as tile
from concourse import bacc
from concourse.bass_utils import run_bass_kernel_spmd
from concourse.masks import make_identity

B, S, E, H, HD = 4, 8192, 2048, 16, 128
SP = S // H            # 512 folded sequence length
NCORES = 8
HPC = H // NCORES      # heads per core = 2
NG = B * HPC           # attention groups per core = 8
ROWS = NG * SP         # projection rows per core = 4096
BROWS = ROWS // B      # rows per batch quarter = 1024
OROWS = B * SP         # output rows = 2048
KC = E // 128          # contraction chunks = 16
JT = SP // 128         # 128-blocks per group = 4
F32 = mybir.dt.float32

# matmul mode: 'f32' (full precision), 'f32r' (1 cyc/row, tf32-like),
# 'bf16', 'f16' (fp16 inputs/projections + f32r attention — halves the DMA
# volume, which is the roofline, at ~6e-4 relative error).
MM_MODE = os.environ.get("MLA_MM_MODE", "f16")

_CACHE = {}
LAST_RESULTS = None  # BassKernelResults of the most recent run (for profiling)


def _build(mm_mode, with_bias):
    # x_dt: dtype of the streamed inputs + projection weights (sets the DMA
    # byte volume).  att_dt: dtype of on-chip latents / attention / out-proj
    # operands.  float32r tiles must be written as float32r end-to-end (BIR
    # rounding rule), hence dtype-typed tiles rather than bitcasts.
    x_dt, att_dt = {
        "f32": (F32, F32),
        "f32r": (mybir.dt.float32r, mybir.dt.float32r),
        "bf16": (mybir.dt.bfloat16, mybir.dt.bfloat16),
        "f16": (mybir.dt.float16, mybir.dt.float32r),
    }[mm_mode]
    # out-projection side (AT, Wo, out partials): fp16 in f16 mode — the
    # partials are summed across cores on the host, so fp16 output
    # quantization stays ~5e-4 while saving ~9MB of DMA per core.
    o_dt = mybir.dt.float16 if mm_mode == "f16" else att_dt
    oo_dt = mybir.dt.float16 if mm_mode == "f16" else F32
    # exp/PV operand side: fp16 halves the E-tile SBUF footprint (deeper
    # input prefetch) at ~1e-4 extra error on the softmax weights.
    e_dt = mybir.dt.float16 if mm_mode == "f16" else att_dt

    nc = bacc.Bacc("TRN2", target_bir_lowering=False, debug=False,
                   num_devices=NCORES)
    # x layout: [128, B, KC, BROWS] flattened — a per-batch 8-chunk strip is
    # one DMA with 16KB+ contiguous partition lines.
    xq = nc.dram_tensor("xq", [128, KC * ROWS], x_dt, kind="ExternalInput")
    xk = nc.dram_tensor("xk", [128, KC * ROWS], x_dt, kind="ExternalInput")
    xv = nc.dram_tensor("xv", [128, KC * ROWS], x_dt, kind="ExternalInput")
    wq = nc.dram_tensor("wq", [128, KC * HD], x_dt, kind="ExternalInput")
    wk = nc.dram_tensor("wk", [128, KC * HD], x_dt, kind="ExternalInput")
    wv = nc.dram_tensor("wv", [128, KC * HD], x_dt, kind="ExternalInput")
    wo = nc.dram_tensor("wo", [128, HPC * E], o_dt, kind="ExternalInput")
    if with_bias:
        bqkv = nc.dram_tensor("bqkv", [3, HD], F32, kind="ExternalInput")
    out = nc.dram_tensor("out", [OROWS, E], oo_dt, kind="ExternalOutput")

    with tile.TileContext(nc) as tc:
        with tc.tile_pool(name="persist", bufs=1) as persist, \
             tc.tile_pool(name="lat", bufs=3) as lat, \
             tc.tile_pool(name="work", bufs=2) as work, \
             tc.tile_pool(name="xin", bufs=7) as xin, \
             tc.tile_pool(name="psproj", bufs=2, space="PSUM") as psproj, \
             tc.tile_pool(name="pss", bufs=2, space="PSUM") as pss, \
             tc.tile_pool(name="pssum", bufs=1, space="PSUM") as pssum, \
             tc.tile_pool(name="pso", bufs=1, space="PSUM") as pso, \
             tc.tile_pool(name="psod", bufs=2, space="PSUM") as psod:
            # all-ones stationary: ones^T @ E gives the softmax denominator
            # replicated across all 128 output partitions (no cross-partition
            # broadcast needed).  memset/affine_select only handle plain
            # dtypes -> build in f32 and convert.
            ones_t = persist.tile([128, 128], e_dt)
            ident = persist.tile([128, 128], att_dt)
            if e_dt == F32:
                nc.gpsimd.memset(ones_t[:], 1.0)
                make_identity(nc, ident[:])
            else:
                scratch = persist.tile([128, 128], F32)
                nc.gpsimd.memset(scratch[:], 1.0)
                nc.vector.tensor_copy(ones_t[:], scratch[:])
                make_identity(nc, scratch[:])
                nc.vector.tensor_copy(ident[:], scratch[:])
            if with_bias:
                bias_sb = persist.tile([128, 3], F32)
                nc.sync.dma_start(bias_sb[:], bqkv[:].rearrange("t l -> l t"))

            w_sb = {}
            for name, w in (("q", wq), ("k", wk), ("v", wv)):
                t = persist.tile([128, KC, HD], x_dt, tag=f"w_{name}",
                                 name=f"w_{name}")
                nc.sync.dma_start(t[:], w[:].rearrange("p (c l) -> p c l",
                                                       c=KC))
                w_sb[name] = t
            wo_sb = persist.tile([128, HPC, E], o_dt)

            # x layout: [128, NG, KC, SP] flattened — one 2MB strip per
            # (tensor, group) with 16KB contiguous partition lines.
            xr = {name: src[:].rearrange("p (g c r) -> p g c r", g=NG, c=KC)
                  for name, src in (("q", xq), ("k", xk), ("v", xv))}

            qTs, kTs, vNs, ATs = {}, {}, {}, {}

            def proj_stage(g):
                """Project q/k/v for group g (rows g*512..), transpose v."""
                lats = {}
                for ti, name in enumerate(("q", "k", "v")):
                    dst = lat.tile([128, SP], att_dt, tag=f"{name}T",
                                   name=f"{name}T_{g}")
                    lats[name] = dst
                    ps = psproj.tile([128, SP], F32, tag="proj",
                                     name=f"ps_{name}_{g}")
                    xs = xin.tile([128, KC, SP], x_dt, tag="xstrip",
                                  name=f"xs_{name}_{g}")
                    # group 0: split the strip load so the first matmuls start
                    # after 1/4 of the data instead of the full 2MB.
                    npieces = 4 if g == 0 else 1
                    cper = KC // npieces
                    for p0 in range(0, KC, cper):
                        nc.sync.dma_start(xs[:, p0:p0 + cper],
                                          xr[name][:, g, p0:p0 + cper])
                    for c in range(KC):
                        nc.tensor.matmul(ps[:], w_sb[name][:, c], xs[:, c],
                                         start=(c == 0), stop=(c == KC - 1))
                    if with_bias:
                        nc.vector.tensor_scalar_add(dst[:], ps[:],
                                                    bias_sb[:, ti:ti + 1])
                    else:
                        nc.vector.tensor_copy(dst[:], ps[:])
                qTs[g], kTs[g] = lats["q"], lats["k"]
                vN = lat.tile([128, JT, HD], e_dt, tag="vN", name=f"vN_{g}")
                vNs[g] = vN
                for j in range(JT):
                    pt = psod.tile([128, 128], att_dt, tag="od",
                                   name=f"tr_{g}_{j}")
                    nc.tensor.transpose(pt[:], lats["v"][:, j * 128:(j + 1) * 128],
                                        ident[:])
                    nc.vector.tensor_copy(vN[:, j], pt[:])

            def attn_stage(g):
                b, hl = divmod(g, HPC)
                if hl == 0:
                    ATs[b] = lat.tile([128, HPC, SP], o_dt, tag="AT",
                                      name=f"AT_{b}")
                Esb = work.tile([128, JT, SP], e_dt, tag="E", name=f"E_{g}")
                for j in range(JT):
                    sp = pss.tile([128, SP], F32, tag="S", name=f"S_{g}_{j}")
                    nc.tensor.matmul(sp[:], kTs[g][:, j * 128:(j + 1) * 128],
                                     qTs[g][:], start=True, stop=True)
                    nc.scalar.activation(Esb[:, j], sp[:],
                                         mybir.ActivationFunctionType.Exp)
                sum_ps = pssum.tile([128, SP], F32, tag="sum", name=f"sum_{g}")
                for j in range(JT):
                    nc.tensor.matmul(sum_ps[:], ones_t[:], Esb[:, j],
                                     start=(j == 0), stop=(j == JT - 1))
                o_ps = pso.tile([128, SP], F32, tag="O", name=f"O_{g}")
                for j in range(JT):
                    nc.tensor.matmul(o_ps[:], vNs[g][:, j], Esb[:, j],
                                     start=(j == 0), stop=(j == JT - 1))
                rec_b = work.tile([128, SP], F32, tag="recb", name=f"rec_{g}")
                # ~51 ULP, ~5x faster than reciprocal() — this op sits on the
                # critical path between the denominator matmul and PV drain.
                # Inputs are softmax sums (>= 1), so no edge cases.
                nc.vector.reciprocal_approx_fast(rec_b[:], sum_ps[:])
                nc.vector.tensor_tensor(ATs[b][:, hl], o_ps[:], rec_b[:],
                                        op=mybir.AluOpType.mult)
                del qTs[g], kTs[g], vNs[g]

            def out_stage(b):
                AT = ATs[b]
                # the last batch's out-projection runs after all input strips
                # are done: the projection PSUM banks are free by then, so
                # borrow them for double the slots (drain throughput).
                pspool = psproj if b == B - 1 else psod
                pstag = "proj" if b == B - 1 else "od"
                for rto in range(SP // 128):
                    ot = work.tile([128, E], oo_dt, tag="ot",
                                   name=f"ot_{b}_{rto}")
                    for n in range(E // 512):
                        ps = pspool.tile([128, 512], F32, tag=pstag,
                                         name=f"od_{b}_{rto}_{n}")
                        for hl in range(HPC):
                            nc.tensor.matmul(
                                ps[:], AT[:, hl, rto * 128:(rto + 1) * 128],
                                wo_sb[:, hl, n * 512:(n + 1) * 512],
                                start=(hl == 0), stop=(hl == HPC - 1))
                        # alternate drains across ACT and DVE so neither
                        # engine rate-limits the PSUM slot turnover
                        dst = ot[:, n * 512:(n + 1) * 512]
                        if n % 2 == 0:
                            nc.scalar.copy(dst, ps[:])
                        else:
                            nc.vector.tensor_copy(dst, ps[:])
                    r0 = b * SP + rto * 128
                    nc.sync.dma_start(out[r0:r0 + 128, :], ot[:])
                del ATs[b]

            # software-pipelined emission: projections run two groups ahead
            # of attention so the input-strip DMA never starves; the serial
            # tail is one group's attention + one out-projection.
            proj_stage(0)
            proj_stage(1)
            nc.sync.dma_start(wo_sb[:],
                              wo[:].rearrange("p (h e) -> p h e", h=HPC))
            for g in range(NG):
                attn_stage(g)
                if g + 2 < NG:
                    proj_stage(g + 2)
                if g % 2 == 1:
                    out_stage(g // 2)

    nc.compile()
    return nc


def _rope_matrix():
    h2 = HD // 2
    freqs = 1.0 / (10000.0 ** (np.arange(0, HD, 2, dtype=np.float64) / HD))
    sin, cos = np.sin(freqs), np.cos(freqs)
    R = np.zeros((HD, HD), np.float64)
    i = np.arange(h2)
    R[i, i] = cos
    R[i + h2, i] = -sin
    R[i + h2, i + h2] = cos
    R[i, i + h2] = sin
    return R


def kernel(query, key, value, attn_mask, Wq, bq, Wk, bk, Wv, bv, Wo, bo,
           _trace=False):
    global LAST_RESULTS
    # inputs may arrive as jax arrays — coerce to host numpy first
    query, key, value = np.asarray(query), np.asarray(key), np.asarray(value)
    Wq, bq = np.asarray(Wq), np.asarray(bq)
    Wk, bk = np.asarray(Wk), np.asarray(bk)
    Wv, bv = np.asarray(Wv), np.asarray(bv)
    Wo, bo = np.asarray(Wo), np.asarray(bo)
    mm_mode = MM_MODE
    io_np = np.dtype("float32")
    wo_np = np.dtype("float32")
    if mm_mode == "bf16":
        import ml_dtypes
        io_np = np.dtype(ml_dtypes.bfloat16)
        wo_np = io_np
    elif mm_mode == "f16":
        io_np = np.dtype("float16")
        wo_np = io_np

    R = _rope_matrix()
    scale = 1.0 / np.sqrt(np.float64(HD))
    wq_eff = (Wq.astype(np.float64) @ R * scale).astype(io_np)
    wk_eff = (Wk.astype(np.float64) @ R).astype(io_np)
    wv_eff = Wv.astype(io_np)
    bq_eff = (bq.astype(np.float64) @ R * scale).astype(np.float32)
    bk_eff = (bk.astype(np.float64) @ R).astype(np.float32)
    bv_eff = bv.astype(np.float32)
    with_bias = bool(np.any(bq_eff) or np.any(bk_eff) or np.any(bv_eff))

    key_ = (mm_mode, with_bias)
    if key_ not in _CACHE:
        _CACHE[key_] = _build(mm_mode, with_bias)
    nc = _CACHE[key_]

    # [B,S,E] -> [E, B, H, SP]; s = s'*H + h so reshape(B, SP, H, E) puts the
    # folded position s' on axis 1 and the head on axis 2.
    def fold(x):
        return np.ascontiguousarray(
            x.reshape(B, SP, H, E).transpose(3, 0, 2, 1).astype(io_np))

    fq, fk, fv = fold(query), fold(key), fold(value)
    wo_r = Wo.reshape(H, HD, E)

    def pmajor(xc):
        # [E, ROWS(b,hl,s')] -> [128, NG, KC, SP] flattened: partition is the
        # inner 128 of the E-chunk; per-group strips contiguous (16KB lines).
        return np.ascontiguousarray(
            xc.reshape(KC, 128, NG, SP).transpose(1, 2, 0, 3)
        ).reshape(128, KC * ROWS)

    # pre-lay weights in SBUF order ([128 partitions, ...]).
    def sb_layout_w(w_eff):  # [E, HD] -> [128, KC*HD]
        return np.ascontiguousarray(
            w_eff.reshape(KC, 128, HD).transpose(1, 0, 2).reshape(128, KC * HD))

    wq_sb, wk_sb, wv_sb = map(sb_layout_w, (wq_eff, wk_eff, wv_eff))

    in_maps = []
    for c in range(NCORES):
        h0 = HPC * c
        wo_c = wo_r[h0:h0 + HPC].astype(wo_np)  # [HPC, HD, E]
        m = {
            "xq": pmajor(fq[:, :, h0:h0 + HPC, :].reshape(E, ROWS)),
            "xk": pmajor(fk[:, :, h0:h0 + HPC, :].reshape(E, ROWS)),
            "xv": pmajor(fv[:, :, h0:h0 + HPC, :].reshape(E, ROWS)),
            "wq": wq_sb, "wk": wk_sb, "wv": wv_sb,
            "wo": np.ascontiguousarray(
                wo_c.transpose(1, 0, 2).reshape(128, HPC * E)),
        }
        if with_bias:
            m["bqkv"] = np.stack([bq_eff, bk_eff, bv_eff])
        in_maps.append(m)

    kwargs = {}
    if _trace:
        kwargs = dict(trace=True, trace_cores=list(range(NCORES)))
    res = run_bass_kernel_spmd(nc, in_maps, core_ids=list(range(NCORES)),
                               **kwargs)
    LAST_RESULTS = res

    total = res.results[0]["out"].astype(np.float64)
    for c in range(1, NCORES):
        total += res.results[c]["out"]
    total += bo.astype(np.float64)
    return total.reshape(B, SP, E).astype(np.float32)
